# revision 3
# baseline (speedup 1.0000x reference)
"""Trainium2 Bass kernel for nn_GATLayer_58291296141986.

Math: the reference computes
    xt = (x @ W.T).reshape(B, N, H, D)            # B=32, N=10, H=8, D=8
    out[b,n,h,m] = relu(sum_k xt[b,n,h,k] * adj[b,n,m])
adj does not depend on k, so sum_k xt[b,n,h,k] = x[b,n,:] @ Wsum[h,:]
with Wsum[h] = sum_d W[h*8+d].  The whole problem collapses to
    s = x2 @ Wsum.T        # (320, 65536) @ (65536, 8)
    out[t, h*10+m] = relu(s[t,h] * adj[t,m])
which is memory-bound on reading x (84MB) + W (17MB).

Sharding: tensor-parallel over in_dim (k).  Each of the 8 cores reads a
disjoint 8192-wide k-slice of x (10.5MB) and W (2MB), computes a partial
s^T (8, 320), then a ReduceScatter(add) over the 8 cores sums the partials
and hands core h the row for head h.  Core h computes its 10 output
columns relu(s[:,h] * adj) for all 320 tokens; the host concatenates the
8 head-slices.  Total HBM read is ~12.6MB/core -- every input byte is read
exactly once across the chip.

Device layout trick: the PE contracts over the partition axis, but x in
DRAM is token-major.  The host pre-swizzles each core's x slice to
    xs[p, j*320 + t] = x2[t, c*8192 + p*64 + j]   (p in 0..128, j in 0..64)
so one fp32 matmul per j (lhsT = Wsum slice (128,8), rhs = xs slice
(128,320)) accumulates s^T over 64 PSUM-accumulated matmuls, with the
xs DMA arriving in 8 j-major chunks that pipeline against the PE.
W is likewise pre-swizzled so an on-device reduce over the innermost
8 (the head's D entries) yields Wsum in exactly the lhsT layout needed.
"""

import numpy as np

import concourse.bass as bass
import concourse.mybir as mybir
import concourse.tile as tile
from concourse import bacc
from concourse.bass_utils import run_bass_kernel_spmd

B, NN, IN_DIM, OUT_DIM, HEADS = 32, 10, 65536, 64, 8
NCORES = 8
T = B * NN                 # 320 tokens
KS = IN_DIM // NCORES      # 8192 contraction slice per core
JW = KS // 128             # 64 j-steps per core
NCHUNK = 8                 # xs DMA chunks
JC = JW // NCHUNK          # j-steps per chunk
F32 = mybir.dt.float32


def build_module():
    nc = bacc.Bacc("TRN2", debug=False, num_devices=NCORES, target_bir_lowering=False)

    xs_d = nc.dram_tensor("xs", [128, JW * T], F32, kind="ExternalInput").ap()
    ws_d = nc.dram_tensor("ws", [128, JW * HEADS * 8], F32, kind="ExternalInput").ap()
    adj_d = nc.dram_tensor("adjt", [NN, T], F32, kind="ExternalInput").ap()
    out_d = nc.dram_tensor("out", [NN, T], F32, kind="ExternalOutput").ap()

    with tile.TileContext(nc) as tc:
        with (
            tc.tile_pool(name="xp", bufs=NCHUNK) as xp,
            tc.tile_pool(name="aux", bufs=1) as aux,
            tc.tile_pool(name="pp", bufs=1, space="PSUM") as pp,
            tc.tile_pool(name="dram", bufs=1, space="DRAM") as dram,
        ):
            # W head-sum: ws[p, (j*8+h)*8 + d] -> wsum[p, j*8+h]
            ws_sb = aux.tile([128, JW * HEADS, 8], F32)
            nc.sync.dma_start(ws_sb[:], ws_d.rearrange("p (a d) -> p a d", d=8))
            wsum = aux.tile([128, JW, HEADS], F32)
            nc.vector.reduce_sum(
                out=wsum.rearrange("p j h -> p (j h)").unsqueeze(2),
                in_=ws_sb[:],
                axis=mybir.AxisListType.X,
            )

            # adj^T, needed only at the tail
            adjt_sb = aux.tile([NN, T], F32)
            nc.sync.dma_start(adjt_sb[:], adj_d[:])

            # main contraction: 64 PSUM-accumulated matmuls
            psum_s = pp.tile([HEADS, T], F32)
            for jc in range(NCHUNK):
                xt = xp.tile([128, JC, T], F32, name=f"xt{jc}", tag="xt")
                nc.sync.dma_start(
                    xt[:],
                    xs_d[:, jc * JC * T : (jc + 1) * JC * T].rearrange(
                        "p (a t) -> p a t", a=JC
                    ),
                )
                for a in range(JC):
                    j = jc * JC + a
                    nc.tensor.matmul(
                        psum_s[:],
                        wsum[:, j, :],
                        xt[:, a, :],
                        start=(j == 0),
                        stop=(j == JW - 1),
                    )

            # partial s^T -> DRAM bounce -> ReduceScatter over the 8 cores.
            # arin rows are heads, so core h receives the summed row of head h.
            s_sbT = aux.tile([HEADS, T], F32)
            nc.vector.tensor_copy(s_sbT[:], psum_s[:])
            arin = dram.tile([HEADS, T], F32)
            arout = dram.tile([1, T], F32)
            nc.sync.dma_start(arin[:], s_sbT[:])
            nc.gpsimd.collective_compute(
                "ReduceScatter",
                mybir.AluOpType.add,
                replica_groups=[list(range(NCORES))],
                ins=[arin[:].opt()],
                outs=[arout[:].opt()],
            )

            # tail: out[m, t] = relu(s_head[t] * adj[t, m]) for this core's head
            srow = aux.tile([1, T], F32)
            nc.sync.dma_start(srow[:], arout[:])
            srow10 = aux.tile([NN, T], F32)
            nc.gpsimd.partition_broadcast(srow10[:], srow[:])
            prod = aux.tile([NN, T], F32)
            nc.vector.tensor_mul(prod[:], srow10[:], adjt_sb[:])
            res = aux.tile([NN, T], F32)
            nc.vector.tensor_relu(res[:], prod[:])
            nc.sync.dma_start(out_d[:], res[:])

    nc.compile()
    return nc


def shard_inputs(x, adj, W):
    """Host-side sharding/layout (pure data movement, no math)."""
    x2 = np.ascontiguousarray(x, dtype=np.float32).reshape(T, IN_DIM)
    # xs[c][p, j*T + t] = x2[t, c*KS + p*JW + j]
    xv = x2.reshape(T, NCORES, 128, JW).transpose(1, 2, 3, 0)  # (c, p, j, t)
    xs_all = np.ascontiguousarray(xv).reshape(NCORES, 128, JW * T)
    # ws[c][p, (j*8+h)*8+d] = W[h*8+d, c*KS + p*JW + j]
    Wv = np.ascontiguousarray(W, dtype=np.float32).reshape(
        HEADS, 8, NCORES, 128, JW
    )
    wv = Wv.transpose(2, 3, 4, 0, 1)  # (c, p, j, h, d)
    ws_all = np.ascontiguousarray(wv).reshape(NCORES, 128, JW * HEADS * 8)
    # adj^T (10, 320), identical on every core
    adjt = np.ascontiguousarray(
        np.asarray(adj, dtype=np.float32).reshape(T, NN).T
    )
    return [
        {"xs": xs_all[c], "ws": ws_all[c], "adjt": adjt} for c in range(NCORES)
    ]


_NC = None


def run(x, adj, W, trace=False, **kw):
    global _NC
    if _NC is None:
        _NC = build_module()
    in_maps = shard_inputs(x, adj, W)
    res = run_bass_kernel_spmd(
        _NC, in_maps, core_ids=list(range(NCORES)), trace=trace, **kw
    )
    # core h produced out[m, t] for head h; assemble (320, 80) token-major
    full = np.empty((T, HEADS * NN), dtype=np.float32)
    for h in range(HEADS):
        full[:, h * NN : (h + 1) * NN] = res.results[h]["out"].T
    return full.reshape(B, NN, HEADS * NN), res


def kernel(x, adj, W):
    out, _ = run(x, adj, W)
    return out


# revision 10
# speedup vs baseline: 1.0572x; 1.0572x over previous
"""Trainium2 Bass kernel for nn_GATLayer_58291296141986.

Math: the reference computes
    xt = (x @ W.T).reshape(B, N, H, D)            # B=32, N=10, H=8, D=8
    out[b,n,h,m] = relu(sum_k xt[b,n,h,k] * adj[b,n,m])
adj does not depend on k, so sum_k xt[b,n,h,k] = x[b,n,:] @ Wsum[h,:]
with Wsum[h] = sum_d W[h*8+d].  The whole problem collapses to
    s = x2 @ Wsum.T        # (320, 65536) @ (65536, 8)
    out[t, h*10+m] = relu(s[t,h] * adj[t,m])
which is memory-bound on reading x (84MB) + W (17MB).

Sharding: tensor-parallel over in_dim (k).  Each of the 8 cores reads a
disjoint 8192-wide k-slice of x (10.5MB) and W (2MB), computes a partial
s^T (8, 320), then an AllGather over the 8 cores exchanges the partials
(10KB each).  Core h folds the 8 partials for head h with a one-hot
matmul (which also broadcasts the row to 10 PSUM partitions), multiplies
by adj^T and applies relu, producing the 10 output columns of head h for
all 320 tokens; the host concatenates the 8 head-slices.  Total HBM read
is ~12.6MB/core -- every input byte is read exactly once across the chip.

Device layout trick: the PE contracts over the partition axis, but x in
DRAM is token-major.  The host pre-swizzles each core's x slice to
    xs[p, j*320 + t] = x2[t, c*8192 + p*64 + j]   (p in 0..128, j in 0..64)
so one matmul per j (lhsT = Wsum slice (128,8), rhs = xs slice (128,320))
accumulates s^T over 64 PSUM-accumulated matmuls, with the xs DMA
arriving in 8 j-major chunks that pipeline against the PE.  W is likewise
pre-swizzled so an on-device reduce over the innermost 8 (the head's D
entries) yields Wsum in exactly the lhsT layout needed.  The matmul
operands are bitcast to float32r (same bytes, single-pass PE matmul at
1 cycle/row for free dim >= 256, vs 4 cycles/row for plain fp32).
"""

import numpy as np

import concourse.bass as bass
import concourse.mybir as mybir
import concourse.tile as tile
from concourse import bacc
from concourse.bass_utils import run_bass_kernel_spmd

B, NN, IN_DIM, OUT_DIM, HEADS = 32, 10, 65536, 64, 8
NCORES = 8
T = B * NN                 # 320 tokens
KS = IN_DIM // NCORES      # 8192 contraction slice per core
JW = KS // 128             # 64 j-steps per core
NCHUNK = 8                 # xs DMA chunks
JC = JW // NCHUNK          # j-steps per chunk
F32 = mybir.dt.float32
F32R = mybir.dt.float32r

def build_module():
    nc = bacc.Bacc("TRN2", debug=False, num_devices=NCORES, target_bir_lowering=False)

    xs_d = nc.dram_tensor("xs", [128, JW * T], F32R, kind="ExternalInput").ap()
    ws_d = nc.dram_tensor("ws", [128, JW * HEADS * 8], F32, kind="ExternalInput").ap()
    adj_d = nc.dram_tensor("adjt", [NN, T], F32, kind="ExternalInput").ap()
    hsel_d = nc.dram_tensor("hsel", [HEADS, NN], F32, kind="ExternalInput").ap()
    out_d = nc.dram_tensor("out", [NN, T], F32, kind="ExternalOutput").ap()

    with tile.TileContext(nc) as tc:
        with (
            tc.tile_pool(name="xp", bufs=NCHUNK) as xp,
            tc.tile_pool(name="wp", bufs=NCHUNK) as wp,
            tc.tile_pool(name="aux", bufs=1) as aux,
            tc.tile_pool(name="pp", bufs=1, space="PSUM") as pp,
            tc.tile_pool(name="dram", bufs=1, space="DRAM") as dram,
        ):
            # tiny tail inputs first (cheap, get them out of the way)
            adjt_sb = aux.tile([NN, T], F32)
            nc.sync.dma_start(adjt_sb[:], adj_d[:])
            hsel_sb = aux.tile([HEADS, NN], F32)
            nc.sync.dma_start(hsel_sb[:], hsel_d[:])

            # warm up the collectives firmware: a tiny throwaway AllGather
            # posted up front absorbs the ncfw cold-start latency so the
            # real collective at the tail starts promptly
            wuin = dram.tile([1, 8], F32)
            wuout = dram.tile([NCORES, 8], F32, addr_space="Shared")
            wu_sb = aux.tile([1, 8], F32)
            nc.vector.memset(wu_sb[:], 0.0)
            nc.sync.dma_start(wuin[:], wu_sb[:])
            nc.gpsimd.collective_compute(
                "AllGather",
                mybir.AluOpType.bypass,
                replica_groups=[list(range(NCORES))],
                ins=[wuin[:].opt()],
                outs=[wuout[:].opt()],
            )

            # main contraction: 64 PSUM-accumulated matmuls, chunk-paced.
            # W head-sum per chunk: ws[p, (j*8+h)*8+d] -> wsum[p, j*8+h]
            psum_s = pp.tile([HEADS, T], F32)
            for jc in range(NCHUNK):
                wst = wp.tile([128, JC * HEADS * 8], F32, name=f"wst{jc}", tag="wst")
                nc.sync.dma_start(
                    wst[:],
                    ws_d[:, jc * JC * HEADS * 8 : (jc + 1) * JC * HEADS * 8],
                )
                wsum = wp.tile([128, JC * HEADS], F32R, name=f"wsum{jc}", tag="wsum")
                with nc.allow_low_precision(reason="f32r rounding of Wsum is the intended matmul precision"):
                    nc.vector.reduce_sum(
                        out=wsum[:].unsqueeze(2),
                        in_=wst[:].rearrange("p (a d) -> p a d", d=8),
                        axis=mybir.AxisListType.X,
                    )
                xt = xp.tile([128, JC * T], F32R, name=f"xt{jc}", tag="xt")
                nc.sync.dma_start(
                    xt[:],
                    xs_d[:, jc * JC * T : (jc + 1) * JC * T],
                )
                for a in range(JC):
                    j = jc * JC + a
                    nc.tensor.matmul(
                        psum_s[:],
                        wsum[:, a * HEADS : (a + 1) * HEADS],
                        xt[:, a * T : (a + 1) * T],
                        start=(j == 0),
                        stop=(j == JW - 1),
                    )

            # partial s^T -> DRAM bounce -> AllToAll: row h of arin is this
            # core's partial for head h; after A2A, core c holds every rank's
            # partial of head c (8 rows), only 10KB on the wire.
            s_sbT = aux.tile([HEADS, T], F32)
            nc.vector.tensor_copy(s_sbT[:], psum_s[:])
            arin = dram.tile([HEADS, T], F32)
            a2aout = dram.tile([HEADS, T], F32)
            nc.sync.dma_start(arin[:], s_sbT[:])
            nc.gpsimd.collective_compute(
                "AllToAll",
                mybir.AluOpType.bypass,
                replica_groups=[list(range(NCORES))],
                ins=[arin[:].opt()],
                outs=[a2aout[:].opt()],
            )

            # fold the 8 partials of this core's head (ones matmul, which
            # also replicates the summed row onto 10 PSUM partitions), then
            # out[m, t] = relu(s_head[t] * adj[t, m])
            ag_sb = aux.tile([HEADS, T], F32)
            nc.sync.dma_start(ag_sb[:], a2aout[:])
            psum10 = pp.tile([NN, T], F32)
            nc.tensor.matmul(
                psum10[:], hsel_sb[:], ag_sb[:], start=True, stop=True
            )
            prod = aux.tile([NN, T], F32)
            nc.vector.tensor_mul(prod[:], psum10[:], adjt_sb[:])
            res = aux.tile([NN, T], F32)
            nc.vector.tensor_relu(res[:], prod[:])
            nc.sync.dma_start(out_d[:], res[:])

    nc.compile()
    return nc


def shard_inputs(x, adj, W):
    """Host-side sharding/layout (pure data movement, no math)."""
    x2 = np.ascontiguousarray(x, dtype=np.float32).reshape(T, IN_DIM)
    # xs[c][p, j*T + t] = x2[t, c*KS + p*JW + j]
    xv = x2.reshape(T, NCORES, 128, JW).transpose(1, 2, 3, 0)  # (c, p, j, t)
    xs_all = np.ascontiguousarray(xv).reshape(NCORES, 128, JW * T)
    # ws[c][p, (j*8+h)*8+d] = W[h*8+d, c*KS + p*JW + j]
    Wv = np.ascontiguousarray(W, dtype=np.float32).reshape(
        HEADS, 8, NCORES, 128, JW
    )
    wv = Wv.transpose(2, 3, 4, 0, 1)  # (c, p, j, h, d)
    ws_all = np.ascontiguousarray(wv).reshape(NCORES, 128, JW * HEADS * 8)
    # adj^T (10, 320), identical on every core
    adjt = np.ascontiguousarray(
        np.asarray(adj, dtype=np.float32).reshape(T, NN).T
    )
    # fold matrix: after the A2A every row of a2aout is a partial of this
    # core's head, so the fold is a plain ones-matmul (same on all cores)
    hsel = np.ones((HEADS, NN), dtype=np.float32)
    return [
        {"xs": xs_all[c], "ws": ws_all[c], "adjt": adjt, "hsel": hsel}
        for c in range(NCORES)
    ]


_NC = None


def run(x, adj, W, trace=False, **kw):
    global _NC
    if _NC is None:
        _NC = build_module()
    in_maps = shard_inputs(x, adj, W)
    res = run_bass_kernel_spmd(
        _NC, in_maps, core_ids=list(range(NCORES)), trace=trace, **kw
    )
    # core h produced out[m, t] for head h; assemble (320, 80) token-major
    full = np.empty((T, HEADS * NN), dtype=np.float32)
    for h in range(HEADS):
        full[:, h * NN : (h + 1) * NN] = res.results[h]["out"].T
    return full.reshape(B, NN, HEADS * NN), res


def kernel(x, adj, W):
    out, _ = run(x, adj, W)
    return out


# revision 12
# speedup vs baseline: 1.4780x; 1.3980x over previous
"""Trainium2 Bass kernel for nn_GATLayer_58291296141986.

Math: the reference computes
    xt = (x @ W.T).reshape(B, N, H, D)            # B=32, N=10, H=8, D=8
    out[b,n,h,m] = relu(sum_k xt[b,n,h,k] * adj[b,n,m])
adj does not depend on k, so sum_k xt[b,n,h,k] = x[b,n,:] @ Wsum[h,:]
with Wsum[h] = sum_d W[h*8+d].  The whole problem collapses to
    s = x2 @ Wsum.T        # (320, 65536) @ (65536, 8)
    out[t, h*10+m] = relu(s[t,h] * adj[t,m])
which is memory-bound on reading x (84MB) + W (17MB).

Sharding: tensor-parallel over in_dim (k).  Each of the 8 cores reads a
disjoint 8192-wide k-slice of x (10.5MB) and W (2MB) and computes a
partial s^T (8, 320) -- every input byte is read exactly once across the
chip (~12.6MB/core, the memory roofline).  The cross-core reduction of
the 10KB partials is done in a second, tiny SPMD launch: the host hands
core h the 8 partial rows of head h (pure data movement), and the device
folds them with a ones-matmul (which also replicates the summed row onto
10 PSUM partitions), multiplies by adj^T and applies relu.  Core h thus
produces the 10 output columns of head h for all 320 tokens and the host
concatenates the 8 head slices.  (A single-launch variant with an
on-device AllToAll was measured ~30us slower: the collectives firmware's
entry barrier alone costs ~60us on this runtime.)

Device layout trick: the PE contracts over the partition axis, but x in
DRAM is token-major.  The host pre-swizzles each core's x slice to
    xs[p, j*320 + t] = x2[t, c*8192 + p*64 + j]   (p in 0..128, j in 0..64)
so one matmul per j (lhsT = Wsum slice (128,8), rhs = xs slice (128,320))
accumulates s^T over 64 PSUM-accumulated matmuls, with the xs DMA
arriving in 8 j-major chunks that pipeline against the PE.  W is likewise
pre-swizzled so an on-device reduce over the innermost 8 (the head's D
entries) yields Wsum in exactly the lhsT layout needed.  Matmul operands
are float32r: same fp32 bytes, single-pass PE matmul at 1 cycle/row
(plain fp32 is 4 cycles/row), costing ~1e-4 relative error.
"""

import numpy as np

import concourse.bass as bass
import concourse.mybir as mybir
import concourse.tile as tile
from concourse import bacc
from concourse.bass_utils import run_bass_kernel_spmd

B, NN, IN_DIM, OUT_DIM, HEADS = 32, 10, 65536, 64, 8
NCORES = 8
T = B * NN                 # 320 tokens
KS = IN_DIM // NCORES      # 8192 contraction slice per core
JW = KS // 128             # 64 j-steps per core
NCHUNK = 8                 # xs DMA chunks
JC = JW // NCHUNK          # j-steps per chunk
F32 = mybir.dt.float32
F32R = mybir.dt.float32r


def build_main():
    """Launch 1: per-core partial s^T = (x k-slice) @ (Wsum k-slice)^T."""
    nc = bacc.Bacc("TRN2", debug=False, num_devices=NCORES, target_bir_lowering=False)

    xs_d = nc.dram_tensor("xs", [128, JW * T], F32R, kind="ExternalInput").ap()
    ws_d = nc.dram_tensor("ws", [128, JW * HEADS * 8], F32, kind="ExternalInput").ap()
    part_d = nc.dram_tensor("part", [HEADS, T], F32, kind="ExternalOutput").ap()

    with tile.TileContext(nc) as tc:
        with (
            tc.tile_pool(name="xp", bufs=NCHUNK) as xp,
            tc.tile_pool(name="wp", bufs=NCHUNK) as wp,
            tc.tile_pool(name="aux", bufs=1) as aux,
            tc.tile_pool(name="pp", bufs=1, space="PSUM") as pp,
        ):
            psum_s = pp.tile([HEADS, T], F32)
            for jc in range(NCHUNK):
                wst = wp.tile([128, JC * HEADS * 8], F32, name=f"wst{jc}", tag="wst")
                nc.sync.dma_start(
                    wst[:],
                    ws_d[:, jc * JC * HEADS * 8 : (jc + 1) * JC * HEADS * 8],
                )
                wsum = wp.tile([128, JC * HEADS], F32R, name=f"wsum{jc}", tag="wsum")
                with nc.allow_low_precision(
                    reason="f32r rounding of Wsum is the intended matmul precision"
                ):
                    nc.vector.reduce_sum(
                        out=wsum[:].unsqueeze(2),
                        in_=wst[:].rearrange("p (a d) -> p a d", d=8),
                        axis=mybir.AxisListType.X,
                    )
                xt = xp.tile([128, JC * T], F32R, name=f"xt{jc}", tag="xt")
                nc.sync.dma_start(
                    xt[:],
                    xs_d[:, jc * JC * T : (jc + 1) * JC * T],
                )
                for a in range(JC):
                    j = jc * JC + a
                    nc.tensor.matmul(
                        psum_s[:],
                        wsum[:, a * HEADS : (a + 1) * HEADS],
                        xt[:, a * T : (a + 1) * T],
                        start=(j == 0),
                        stop=(j == JW - 1),
                    )

            s_sbT = aux.tile([HEADS, T], F32)
            nc.vector.tensor_copy(s_sbT[:], psum_s[:])
            nc.sync.dma_start(part_d[:], s_sbT[:])

    nc.compile()
    return nc


def build_fold():
    """Launch 2: core h folds head h's 8 partials, scales by adj^T, relu."""
    nc = bacc.Bacc("TRN2", debug=False, num_devices=NCORES, target_bir_lowering=False)

    parts_d = nc.dram_tensor("parts", [NCORES, T], F32, kind="ExternalInput").ap()
    adj_d = nc.dram_tensor("adjt", [NN, T], F32, kind="ExternalInput").ap()
    ones_d = nc.dram_tensor("ones", [NCORES, NN], F32, kind="ExternalInput").ap()
    out_d = nc.dram_tensor("out", [NN, T], F32, kind="ExternalOutput").ap()

    with tile.TileContext(nc) as tc:
        with (
            tc.tile_pool(name="aux", bufs=1) as aux,
            tc.tile_pool(name="pp", bufs=1, space="PSUM") as pp,
        ):
            parts_sb = aux.tile([NCORES, T], F32)
            nc.sync.dma_start(parts_sb[:], parts_d[:])
            adjt_sb = aux.tile([NN, T], F32)
            nc.sync.dma_start(adjt_sb[:], adj_d[:])
            ones_sb = aux.tile([NCORES, NN], F32)
            nc.sync.dma_start(ones_sb[:], ones_d[:])

            # ones-matmul: sums the 8 partial rows and replicates the sum
            # onto 10 PSUM partitions in one shot
            psum10 = pp.tile([NN, T], F32)
            nc.tensor.matmul(psum10[:], ones_sb[:], parts_sb[:], start=True, stop=True)
            prod = aux.tile([NN, T], F32)
            nc.vector.tensor_mul(prod[:], psum10[:], adjt_sb[:])
            res = aux.tile([NN, T], F32)
            nc.vector.tensor_relu(res[:], prod[:])
            nc.sync.dma_start(out_d[:], res[:])

    nc.compile()
    return nc


def shard_inputs(x, adj, W):
    """Host-side sharding/layout (pure data movement, no math)."""
    x2 = np.ascontiguousarray(x, dtype=np.float32).reshape(T, IN_DIM)
    # xs[c][p, j*T + t] = x2[t, c*KS + p*JW + j]
    xv = x2.reshape(T, NCORES, 128, JW).transpose(1, 2, 3, 0)  # (c, p, j, t)
    xs_all = np.ascontiguousarray(xv).reshape(NCORES, 128, JW * T)
    # ws[c][p, (j*8+h)*8+d] = W[h*8+d, c*KS + p*JW + j]
    Wv = np.ascontiguousarray(W, dtype=np.float32).reshape(HEADS, 8, NCORES, 128, JW)
    wv = Wv.transpose(2, 3, 4, 0, 1)  # (c, p, j, h, d)
    ws_all = np.ascontiguousarray(wv).reshape(NCORES, 128, JW * HEADS * 8)
    return [{"xs": xs_all[c], "ws": ws_all[c]} for c in range(NCORES)]


_NC_MAIN = None
_NC_FOLD = None


def run(x, adj, W, trace=False, **kw):
    global _NC_MAIN, _NC_FOLD
    if _NC_MAIN is None:
        _NC_MAIN = build_main()
        _NC_FOLD = build_fold()

    res1 = run_bass_kernel_spmd(
        _NC_MAIN, shard_inputs(x, adj, W), core_ids=list(range(NCORES)),
        trace=trace, **kw
    )
    # host gather/scatter of the 10KB partials: core h gets row h of every
    # core's partial s^T (pure data movement)
    parts = np.stack([res1.results[c]["part"] for c in range(NCORES)])  # (c, h, t)
    adjt = np.ascontiguousarray(np.asarray(adj, dtype=np.float32).reshape(T, NN).T)
    ones = np.ones((NCORES, NN), dtype=np.float32)
    in_maps2 = [
        {"parts": np.ascontiguousarray(parts[:, h, :]), "adjt": adjt, "ones": ones}
        for h in range(HEADS)
    ]
    res2 = run_bass_kernel_spmd(
        _NC_FOLD, in_maps2, core_ids=list(range(NCORES)), trace=trace, **kw
    )

    full = np.empty((T, HEADS * NN), dtype=np.float32)
    for h in range(HEADS):
        full[:, h * NN : (h + 1) * NN] = res2.results[h]["out"].T
    return full.reshape(B, NN, HEADS * NN), (res1, res2)


def kernel(x, adj, W):
    out, _ = run(x, adj, W)
    return out


# revision 17
# speedup vs baseline: 1.8039x; 1.2205x over previous
"""Trainium2 Bass kernel for nn_GATLayer_58291296141986.

Math: the reference computes
    xt = (x @ W.T).reshape(B, N, H, D)            # B=32, N=10, H=8, D=8
    out[b,n,h,m] = relu(sum_k xt[b,n,h,k] * adj[b,n,m])
adj does not depend on k, so sum_k xt[b,n,h,k] = x[b,n,:] @ Wsum[h,:]
with Wsum[h] = sum_d W[h*8+d].  The whole problem collapses to
    s = x2 @ Wsum.T        # (320, 65536) @ (65536, 8)
    out[t, h*10+m] = relu(s[t,h] * adj[t,m])
which is memory-bound on reading x (84MB) + W (17MB).

Sharding: tensor-parallel over in_dim (k).  Each of the 8 cores reads a
disjoint 8192-wide k-slice of x (10.5MB) and W (2MB) and computes a
partial s^T (8, 320) -- every input byte is read exactly once across the
chip (~12.6MB/core, the memory roofline).  The cross-core reduction of
the 10KB partials is done in a second, tiny SPMD launch: the host hands
core h the 8 partial rows of head h (pure data movement), and the device
folds them with a ones-matmul (which also replicates the summed row onto
10 PSUM partitions), multiplies by adj^T and applies relu.  Core h thus
produces the 10 output columns of head h for all 320 tokens and the host
concatenates the 8 head slices.  (A single-launch variant with an
on-device AllToAll was measured ~30us slower: the collectives firmware's
entry barrier alone costs ~60us on this runtime.)

Device layout trick: the PE contracts over the partition axis, but x in
DRAM is token-major.  The host pre-swizzles each core's x slice to
    xs[p, j*320 + t] = x2[t, c*8192 + p*64 + j]   (p in 0..128, j in 0..64)
so one matmul per j (lhsT = Wsum slice (128,8), rhs = xs slice (128,320))
accumulates s^T over 64 PSUM-accumulated matmuls, with the xs DMA
arriving in 8 j-major chunks that pipeline against the PE.  W is likewise
pre-swizzled so an on-device reduce over the innermost 8 (the head's D
entries) yields Wsum in exactly the lhsT layout needed.  Matmul operands
are float32r: same fp32 bytes, single-pass PE matmul at 1 cycle/row
(plain fp32 is 4 cycles/row), costing ~1e-4 relative error.
"""

import numpy as np

import concourse.bass as bass
import concourse.mybir as mybir
import concourse.tile as tile
from concourse import bacc
from concourse.bass_utils import run_bass_kernel_spmd

B, NN, IN_DIM, OUT_DIM, HEADS = 32, 10, 65536, 64, 8
NCORES = 8
T = B * NN                 # 320 tokens
KS = IN_DIM // NCORES      # 8192 contraction slice per core
JW = KS // 128             # 64 j-steps per core
NCHUNK = 8                 # xs DMA chunks
JC = JW // NCHUNK          # j-steps per chunk
F32 = mybir.dt.float32
F32R = mybir.dt.float32r


def build_main():
    """Launch 1: per-core partial s^T = (x k-slice) @ (Wsum k-slice)^T."""
    nc = bacc.Bacc("TRN2", debug=False, num_devices=NCORES, target_bir_lowering=False)

    xs_d = nc.dram_tensor("xs", [128, JW * T], F32R, kind="ExternalInput").ap()
    ws_d = nc.dram_tensor("ws", [128, JW * HEADS * 8], F32, kind="ExternalInput").ap()
    part_d = nc.dram_tensor("part", [HEADS, T], F32, kind="ExternalOutput").ap()

    with tile.TileContext(nc) as tc:
        with (
            tc.tile_pool(name="xp", bufs=NCHUNK) as xp,
            tc.tile_pool(name="wp", bufs=1) as wp,
            tc.tile_pool(name="aux", bufs=1) as aux,
            tc.tile_pool(name="pp", bufs=1, space="PSUM") as pp,
        ):
            # W first (one DMA + one head-sum reduce); Wsum is small and only
            # gates the matmuls, which are chunk-paced by the xs DMAs anyway
            wst = wp.tile([128, JW * HEADS * 8], F32)
            nc.scalar.dma_start(wst[:], ws_d[:])
            wsum = wp.tile([128, JW * HEADS], F32R)
            with nc.allow_low_precision(
                reason="f32r rounding of Wsum is the intended matmul precision"
            ):
                nc.vector.reduce_sum(
                    out=wsum[:].unsqueeze(2),
                    in_=wst[:].rearrange("p (a d) -> p a d", d=8),
                    axis=mybir.AxisListType.X,
                )

            # xs chunks alternate between the two HWDGE rings (SP and ACT)
            # so descriptor generation is not serialized on one engine
            psum_s = pp.tile([HEADS, T], F32)
            for jc in range(NCHUNK):
                xt = xp.tile([128, JC * T], F32R, name=f"xt{jc}", tag="xt")
                eng = nc.sync if jc % 2 == 0 else nc.scalar
                eng.dma_start(
                    xt[:],
                    xs_d[:, jc * JC * T : (jc + 1) * JC * T],
                )
                for a in range(JC):
                    j = jc * JC + a
                    nc.tensor.matmul(
                        psum_s[:],
                        wsum[:, j * HEADS : (j + 1) * HEADS],
                        xt[:, a * T : (a + 1) * T],
                        start=(j == 0),
                        stop=(j == JW - 1),
                    )

            s_sbT = aux.tile([HEADS, T], F32)
            nc.vector.tensor_copy(s_sbT[:], psum_s[:])
            nc.sync.dma_start(part_d[:], s_sbT[:])

    nc.compile()
    return nc


def build_fold():
    """Launch 2: core h folds head h's 8 partials, scales by adj^T, relu."""
    nc = bacc.Bacc("TRN2", debug=False, num_devices=NCORES, target_bir_lowering=False)

    # one merged input: rows 0-7 = the 8 partials of this core's head,
    # rows 32-41 = adj^T (at partition 32: engine APs need base 0/32/64)
    fin_d = nc.dram_tensor("fin", [32 + NN, T], F32, kind="ExternalInput").ap()
    out_d = nc.dram_tensor("out", [NN, T], F32, kind="ExternalOutput").ap()

    with tile.TileContext(nc) as tc:
        with (
            tc.tile_pool(name="aux", bufs=1) as aux,
            tc.tile_pool(name="pp", bufs=1, space="PSUM") as pp,
        ):
            fin_sb = aux.tile([32 + NN, T], F32)
            nc.sync.dma_start(fin_sb[:], fin_d[:])
            ones_sb = aux.tile([NCORES, NN], F32)
            nc.vector.memset(ones_sb[:], 1.0)

            # ones-matmul: sums the 8 partial rows and replicates the sum
            # onto 10 PSUM partitions in one shot
            psum10 = pp.tile([NN, T], F32)
            nc.tensor.matmul(
                psum10[:], ones_sb[:], fin_sb[:NCORES, :], start=True, stop=True
            )
            prod = aux.tile([NN, T], F32)
            nc.vector.tensor_mul(prod[:], psum10[:], fin_sb[32:, :])
            res = aux.tile([NN, T], F32)
            nc.vector.tensor_relu(res[:], prod[:])
            nc.sync.dma_start(out_d[:], res[:])

    nc.compile()
    return nc


def shard_inputs(x, adj, W):
    """Host-side sharding/layout (pure data movement, no math)."""
    x2 = np.ascontiguousarray(x, dtype=np.float32).reshape(T, IN_DIM)
    # xs[c][p, j*T + t] = x2[t, c*KS + p*JW + j]
    xv = x2.reshape(T, NCORES, 128, JW).transpose(1, 2, 3, 0)  # (c, p, j, t)
    xs_all = np.ascontiguousarray(xv).reshape(NCORES, 128, JW * T)
    # ws[c][p, (j*8+h)*8+d] = W[h*8+d, c*KS + p*JW + j]
    Wv = np.ascontiguousarray(W, dtype=np.float32).reshape(HEADS, 8, NCORES, 128, JW)
    wv = Wv.transpose(2, 3, 4, 0, 1)  # (c, p, j, h, d)
    ws_all = np.ascontiguousarray(wv).reshape(NCORES, 128, JW * HEADS * 8)
    return [{"xs": xs_all[c], "ws": ws_all[c]} for c in range(NCORES)]


_NC_MAIN = None
_NC_FOLD = None


def run(x, adj, W, trace=False, **kw):
    global _NC_MAIN, _NC_FOLD
    if _NC_MAIN is None:
        _NC_MAIN = build_main()
        _NC_FOLD = build_fold()

    res1 = run_bass_kernel_spmd(
        _NC_MAIN, shard_inputs(x, adj, W), core_ids=list(range(NCORES)),
        trace=trace, **kw
    )
    # host gather/scatter of the 10KB partials: core h gets row h of every
    # core's partial s^T (pure data movement)
    parts = np.stack([res1.results[c]["part"] for c in range(NCORES)])  # (c, h, t)
    adjt = np.asarray(adj, dtype=np.float32).reshape(T, NN).T
    in_maps2 = []
    for h in range(HEADS):
        fin = np.zeros((32 + NN, T), dtype=np.float32)
        fin[:NCORES] = parts[:, h, :]
        fin[32:] = adjt
        in_maps2.append({"fin": fin})
    res2 = run_bass_kernel_spmd(
        _NC_FOLD, in_maps2, core_ids=list(range(NCORES)), trace=trace, **kw
    )

    full = np.empty((T, HEADS * NN), dtype=np.float32)
    for h in range(HEADS):
        full[:, h * NN : (h + 1) * NN] = res2.results[h]["out"].T
    return full.reshape(B, NN, HEADS * NN), (res1, res2)


def kernel(x, adj, W):
    out, _ = run(x, adj, W)
    return out
